# revision 8
# baseline (speedup 1.0000x reference)
# Trainium2 Bass kernel for BloomStageLoss:
#   loss = mean(label-smoothing CE) + 0.1 * mean(transition penalty)
# over inputs [B, 5] f32, targets [B] int.  B = 4194304, 8 NeuronCores.
#
# Host-side stable sort of rows by target class with bucket-pure
# (core, partition) slots, plus a per-slot class-position permutation
# that puts the diagonal class (T[b,b]=0) last so the penalty pass can
# skip it entirely.
#
# Device math per row i (bucket b, softmax P = e/S):
#   lse_i  = ln S_i,  S_i = sum_c e_ic
#   pen_i  = sum_c T[b,c] P_ic   (diagonal class contributes 0)
# Device pipeline per tile (pos-blocked layout [P, 5*wn], all bf16 in):
#   exp:  DVE 1-instr Schraudolph (bf16 -> int16 RNE, bits are bf16 of
#         e^x) or ACT exp for one tile (ACT has slack)
#   S:    5 accumulating identity matmuls -> PSUM f32
#   Ln:   ACT psS -> lnS bf16 + accum lse
#   rb:   DVE Schraudolph exp(-lnS) ~ 1/S  (1 instr)
#   pen:  either U-matmuls (4 per chunk, diag(T) stationary; U.rb via
#         custom TTR from PSUM) or 3 skip-diag TTRs on e directly.
# A dummy-matmul warmup spin flips the PE HAM clock gate to 2.4 GHz
# before the first real matmul.  Host folds: exact f64 linear terms,
# per-(partition,pos) T-fold of the TTR accums, analytic pad-row
# corrections.

import os
import sys

sys.path.insert(0, "/opt/trn_rl_repo")

import numpy as np
import ml_dtypes
from contextlib import ExitStack

import concourse.bass as bass
import concourse.bacc as bacc
import concourse.tile as tile
from concourse import mybir
from concourse.bass_utils import run_bass_kernel_spmd
from concourse.dve_ops import TENSOR_TENSOR_REDUCE as TTR_OP

NCORES = 8
C = 5
P = 128
B = 4194304
RPP = 4160                      # rows per partition (slot size)
NSLOTS = NCORES * P             # 1024
CAP = NSLOTS * RPP              # 4259840
W_LIST = [256, 1024, 1024, 512, 512, 416, 256, 160]
assert sum(W_LIST) == RPP
TILES = len(W_LIST)
# per-tile engine flags (tunable):
EXP_FLAGS = ["dve", "dve", "act", "dve", "dve", "dve", "dve", "dve"]
PEN_FLAGS = ["ttr", "umm", "umm", "umm", "umm", "umm", "ttr", "umm"]
N_WARMUP_MM = 16                # dummy matmuls to flip the PE HAM gate
# which hardware DGE ring (trigger engine) each x-tile DMA uses
DMA_ENG = ["sync", "scalar", "scalar", "sync", "scalar", "sync", "scalar", "sync"]
SMOOTH_OFF = 0.875              # 1 - SMOOTHING - SMOOTHING/(C-1)
SMOOTH_ALL = 0.025              # SMOOTHING/(C-1)
TPEN = 0.1

_PHI = np.array([0.0, 0.5, 1.0, 2.0, 2.0], dtype=np.float64)
T_MAT = _PHI[np.abs(np.arange(C)[:, None] - np.arange(C)[None, :])]
T_ROWSUM = T_MAT.sum(axis=1)    # [5.5, 4, 3, 4, 5.5]

# class-position permutation per bucket: positions 0,1 singles, 2-3 the
# equal-T pair, 4 the diagonal class (skipped by the pen pass).
PERM = {
    0: [1, 2, 3, 4, 0],
    1: [3, 4, 0, 2, 1],
    2: [1, 3, 0, 4, 2],
    3: [0, 1, 2, 4, 3],
    4: [2, 3, 0, 1, 4],
}
PERM_ARR = np.array([PERM[b] for b in range(C)], dtype=np.int64)   # [5,5]
# fold coefficient per (bucket, pos): T[b, PERM[b][pos]]
FOLD_ARR = np.array(
    [[T_MAT[b, PERM[b][pos]] for pos in range(C)] for b in range(C)]
)
for b in range(C):
    assert FOLD_ARR[b, 2] == FOLD_ARR[b, 3], (b, FOLD_ARR[b])
    assert FOLD_ARR[b, 4] == 0.0

BF16 = ml_dtypes.bfloat16
FP8 = ml_dtypes.float8_e4m3fn

LOG2E = 1.4426950408889634
SCHR_A = 128.0 * LOG2E               # 184.665
SCHR_SHIFT = 7.25                    # mean-centers the sawtooth for N(0,1)
SCHR_B = 16256.0 - SCHR_SHIFT


def _schr_np(x):
    """Host replica of the device Schraudolph exp (bf16-bit space)."""
    i = np.rint(np.asarray(x, np.float32) * SCHR_A + SCHR_B).astype(np.int16)
    return i.view(BF16).astype(np.float64)


_TABLES_PATCHED = False


def _pin_act_tables():
    """Keep Exp/Ln only in their shared set so one ACT table load serves both."""
    global _TABLES_PATCHED
    if _TABLES_PATCHED:
        return
    import concourse.bacc as bacc_mod
    AF = mybir.ActivationFunctionType
    orig = bacc_mod.get_activation_tables

    def patched(arch):
        t = {k: set(v) for k, v in orig(arch).items()}
        both = [k for k, v in t.items() if AF.Exp in v and AF.Ln in v]
        if both:
            keep = both[0]
            for k, v in t.items():
                if k != keep:
                    v.discard(AF.Exp)
                    v.discard(AF.Ln)
        return t

    bacc_mod.get_activation_tables = patched
    _TABLES_PATCHED = True


N_TTR = sum(1 for f in PEN_FLAGS if f == "ttr")
N_UMM = TILES - N_TTR


def build_nc(ncores=NCORES):
    """Build + compile the single-core program (SPMD across ncores)."""
    _pin_act_tables()
    f32 = mybir.dt.float32
    bf16 = mybir.dt.bfloat16
    i16 = mybir.dt.int16
    fp8 = mybir.dt.float8e4
    AF = mybir.ActivationFunctionType
    ALU = mybir.AluOpType

    nc = bacc.Bacc("TRN2", target_bir_lowering=False, debug=False,
                   num_devices=ncores)
    wbf = 5 * sum(w for w, f in zip(W_LIST, EXP_FLAGS) if f == "dve")
    wf8 = 5 * sum(w for w, f in zip(W_LIST, EXP_FLAGS) if f == "act")
    xbf_d = nc.dram_tensor("x_bf", [P, max(wbf, 1)], bf16,
                           kind="ExternalInput").ap()
    xf8_d = nc.dram_tensor("x_f8", [P, max(wf8, 1)], fp8,
                           kind="ExternalInput").ap()
    idn_d = nc.dram_tensor("idn", [P, P], bf16, kind="ExternalInput").ap()
    dgs_d = nc.dram_tensor("dgs", [P, 4 * P], bf16, kind="ExternalInput").ap()
    lse_d = nc.dram_tensor("lse_acc", [P, TILES], f32,
                           kind="ExternalOutput").ap()
    ttr_d = nc.dram_tensor("ttr_acc", [P, max(3 * N_TTR, 1)], f32,
                           kind="ExternalOutput").ap()
    u_d = nc.dram_tensor("u_acc", [P, max(N_UMM, 1)], f32,
                         kind="ExternalOutput").ap()

    with tile.TileContext(nc) as tc, ExitStack() as ctx:
        xpool = ctx.enter_context(tc.tile_pool(name="xp", bufs=1))
        epool = ctx.enter_context(tc.tile_pool(name="ep", bufs=1))
        lpool = ctx.enter_context(tc.tile_pool(name="lp", bufs=1))
        rpool = ctx.enter_context(tc.tile_pool(name="rp", bufs=1))
        spool = ctx.enter_context(tc.tile_pool(name="sp", bufs=1))
        cpool = ctx.enter_context(tc.tile_pool(name="cp", bufs=1))
        apool = ctx.enter_context(tc.tile_pool(name="ap", bufs=1))
        psS_pool = ctx.enter_context(tc.tile_pool(name="psS", bufs=2,
                                                  space="PSUM"))
        psU_pool = ctx.enter_context(tc.tile_pool(name="psU", bufs=2,
                                                  space="PSUM"))

        lse_acc = apool.tile([P, TILES], f32)
        ttr_acc = apool.tile([P, max(3 * N_TTR, 1)], f32)
        u_acc = apool.tile([P, max(N_UMM, 1)], f32)

        bf_off = [0]
        f8_off = [0]
        for w, f in zip(W_LIST, EXP_FLAGS):
            bf_off.append(bf_off[-1] + (5 * w if f == "dve" else 0))
            f8_off.append(f8_off[-1] + (5 * w if f == "act" else 0))

        xts = [None] * TILES
        ets = [None] * TILES
        psSs = [None] * TILES
        psUs = [None] * TILES
        rbs = [None] * TILES
        ttr_idx = 0
        umm_idx = 0

        def dma_tile(n):
            wn = W_LIST[n]
            eng = nc.sync if DMA_ENG[n] == "sync" else nc.scalar
            if EXP_FLAGS[n] == "dve":
                xt = xpool.tile([P, 5 * wn], bf16, tag=f"xb{n}", name="xt")
                eng.dma_start(xt[:], xbf_d[:, bf_off[n]:bf_off[n + 1]])
            else:
                xt = xpool.tile([P, 5 * wn], fp8, tag=f"x8{n}", name="xt")
                eng.dma_start(xt[:], xf8_d[:, f8_off[n]:f8_off[n + 1]])
            xts[n] = xt

        # constants + ALL x tiles triggered up front on two HWDGE rings;
        # per-tile dedicated buffers mean no consumer pacing.
        ident = cpool.tile([P, P], bf16)
        nc.sync.dma_start(ident[:], idn_d)
        for n in range(TILES):
            dma_tile(n)
        dgs = cpool.tile([P, 4 * P], bf16)
        nc.sync.dma_start(dgs[:], dgs_d)

        # PE HAM warmup: dummy matmuls on ident into psS(0)'s region (the
        # real S-matmuls reset it with start=True afterwards).
        psS0 = psS_pool.tile([P, W_LIST[0]], f32, tag="psS")
        psSs[0] = psS0
        with tc.high_priority(offset=300):
            for k in range(N_WARMUP_MM):
                nc.tensor.matmul(psS0[:, 0:P], ident[:], ident[:],
                                 start=(k == 0), stop=(k == N_WARMUP_MM - 1),
                                 skip_group_check=True)

        def exp_tile(n):
            """exp of tile n (engine per EXP_FLAGS)."""
            wn = W_LIST[n]
            xt = xts[n]
            if EXP_FLAGS[n] == "dve":
                et_i = epool.tile([P, 5 * wn], i16, tag=f"ei{n}")
                with tc.high_priority(offset=100):
                    nc.vector.tensor_scalar(et_i[:], xt[:], SCHR_A, SCHR_B,
                                            ALU.mult, ALU.add)
                ets[n] = et_i[:].bitcast(bf16)
            else:
                et_t = epool.tile([P, 5 * wn], bf16, tag=f"eb{n}")
                x3 = xt[:].rearrange("p (c w) -> p c w", c=C)
                e3 = et_t[:].rearrange("p (c w) -> p c w", c=C)
                for j0 in range(0, wn, 512):
                    j1 = min(j0 + 512, wn)
                    nc.scalar.activation(e3[:, :, j0:j1], x3[:, :, j0:j1],
                                         AF.Exp)
                ets[n] = et_t[:]

        def smm_tile(n):
            """S-matmuls (+U-matmuls) of tile n."""
            wn = W_LIST[n]
            et = ets[n]
            if psSs[n] is None:
                psSs[n] = psS_pool.tile([P, wn], f32, tag="psS", name="psS")
            psS = psSs[n]
            for j0 in range(0, wn, 512):
                j1 = min(j0 + 512, wn)
                with tc.high_priority(offset=200):
                    for pos in range(C):
                        nc.tensor.matmul(psS[:, j0:j1], ident[:],
                                         et[:, pos * wn + j0:pos * wn + j1],
                                         start=(pos == 0), stop=(pos == C - 1))
            if PEN_FLAGS[n] == "umm":
                psU = psU_pool.tile([P, wn], f32, tag="psU")
                psUs[n] = psU
                for j0 in range(0, wn, 512):
                    j1 = min(j0 + 512, wn)
                    for pos in range(4):
                        nc.tensor.matmul(psU[:, j0:j1],
                                         dgs[:, pos * P:(pos + 1) * P],
                                         et[:, pos * wn + j0:pos * wn + j1],
                                         start=(pos == 0), stop=(pos == 3))

        def front(n):
            exp_tile(n)
            smm_tile(n)

        def mid(n):
            """Ln + rexp of tile n."""
            wn = W_LIST[n]
            lnS = lpool.tile([P, wn], bf16, tag=f"ln{n}")
            nc.scalar.activation(lnS[:], psSs[n][:], AF.Ln,
                                 accum_out=lse_acc[:, n:n + 1])
            rb_i = rpool.tile([P, wn], i16, tag=f"rb{n}")
            with tc.high_priority(offset=100):
                nc.vector.tensor_scalar(rb_i[:], lnS[:], -SCHR_A, SCHR_B,
                                        ALU.mult, ALU.add)
            rbs[n] = rb_i[:].bitcast(bf16)

        def tail(n):
            """pen accumulation of tile n."""
            nonlocal ttr_idx, umm_idx
            wn = W_LIST[n]
            et = ets[n]
            rb = rbs[n]
            if PEN_FLAGS[n] == "umm":
                scr = spool.tile([P, wn], bf16, tag="scU")
                nc.vector._custom_dve(
                    TTR_OP, out=scr[:], in0=psUs[n][:], in1=rb,
                    s0=0.0, s1=1.0,
                    accum_out=u_acc[:, umm_idx:umm_idx + 1])
                umm_idx += 1
            else:
                k = 3 * ttr_idx
                scr = spool.tile([P, 2 * wn], bf16, tag="scT")
                for pos in range(2):
                    nc.vector._custom_dve(
                        TTR_OP, out=scr[:, pos * wn:(pos + 1) * wn],
                        in0=et[:, pos * wn:(pos + 1) * wn], in1=rb,
                        s0=0.0, s1=1.0,
                        accum_out=ttr_acc[:, k + pos:k + pos + 1])
                # pair: positions 2-3 share the fold coefficient
                e3 = et[:, 2 * wn:4 * wn].rearrange("p (c w) -> p c w", c=2)
                r3 = rb.unsqueeze(1).broadcast_to([P, 2, wn])
                s3 = scr[:].rearrange("p (c w) -> p c w", c=2)
                nc.vector._custom_dve(
                    TTR_OP, out=s3, in0=e3, in1=r3,
                    s0=0.0, s1=1.0,
                    accum_out=ttr_acc[:, k + 2:k + 3])
                ttr_idx += 1

        # software pipeline (DMAs already queued)
        front(0)
        mid(0)
        front(1)
        tail(0)
        mid(1)
        front(2)
        tail(1)
        mid(2)
        front(3)
        tail(2)
        mid(3)
        front(4)
        tail(3)
        mid(4)
        front(5)
        tail(4)
        mid(5)
        front(6)
        tail(5)
        mid(6)
        front(7)
        tail(6)
        mid(7)
        tail(7)

        nc.sync.dma_start(lse_d, lse_acc[:])
        nc.sync.dma_start(ttr_d, ttr_acc[:])
        nc.sync.dma_start(u_d, u_acc[:])

    nc.compile()
    return nc


def _prep_inputs(x: np.ndarray, t: np.ndarray):
    """Sort rows by target, pad buckets to slot (RPP) multiples, apply
    per-slot class-position permutation, lay out pos-blocked per tile."""
    counts = np.bincount(t, minlength=C).astype(np.int64)
    order = np.argsort(t, kind="stable")
    xs = x[order]                               # [B, 5] f32, bucket-contiguous

    # exact host-side sums (f64)
    sum_x = float(x.sum(dtype=np.float64))
    sel_sum = 0.0
    cstart = np.concatenate([[0], np.cumsum(counts)])
    for b in range(C):
        sel_sum += float(xs[cstart[b]:cstart[b + 1], b].sum(dtype=np.float64))

    slots_b = np.ceil(counts / RPP).astype(np.int64)
    assert slots_b.sum() <= NSLOTS, (counts, slots_b)
    slot_start = np.concatenate([[0], np.cumsum(slots_b)])
    # slot -> bucket map; trailing unused slots assigned to bucket C-1
    slot_bucket = np.full(NSLOTS, C - 1, dtype=np.int64)
    for b in range(C):
        slot_bucket[slot_start[b]:slot_start[b + 1]] = b

    # fill count per slot (rows of real data in that slot)
    fill = np.zeros(NSLOTS, dtype=np.int64)
    for b in range(C):
        cnt = counts[b]
        for s in range(slot_start[b], slot_start[b + 1]):
            fill[s] = min(RPP, cnt)
            cnt -= fill[s]

    # padded array [CAP, 5], zero rows as pad
    xpad = np.zeros((CAP, C), dtype=np.float32)
    for b in range(C):
        dst0 = slot_start[b] * RPP
        xpad[dst0:dst0 + counts[b]] = xs[cstart[b]:cstart[b + 1]]

    # per-slot class permutation -> position-blocked
    x3 = xpad.reshape(NSLOTS, RPP, C)
    perm_idx = PERM_ARR[slot_bucket]            # [NSLOTS, 5]
    x3p = np.take_along_axis(x3, perm_idx[:, None, :], axis=2)

    # device layout per tile: [NSLOTS, 5*wn], pos-major
    offs = np.concatenate([[0], np.cumsum(W_LIST)]).astype(int)
    bf_parts, f8_parts = [], []
    for n, wn in enumerate(W_LIST):
        blk = x3p[:, offs[n]:offs[n + 1], :].transpose(0, 2, 1)  # [S, 5, wn]
        blk = blk.reshape(NSLOTS, C * wn)
        if EXP_FLAGS[n] == "dve":
            bf_parts.append(blk.astype(BF16))
        else:
            f8_parts.append(np.clip(blk, -15.0, 15.0).astype(FP8))
    dev_bf = (np.ascontiguousarray(np.concatenate(bf_parts, axis=1))
              if bf_parts else np.zeros((NSLOTS, 1), dtype=BF16))
    dev_f8 = (np.ascontiguousarray(np.concatenate(f8_parts, axis=1))
              if f8_parts else np.zeros((NSLOTS, 1), dtype=FP8))

    # per-slot diag values for U-matmuls (positions 0..3)
    dvals = FOLD_ARR[slot_bucket][:, :4]        # [NSLOTS, 4]
    # fold coefficients for ttr tiles: [NSLOTS, 3] (single, single, pair)
    folds = FOLD_ARR[slot_bucket][:, [0, 1, 2]]

    per_core = []
    for k in range(NCORES):
        sl = slice(k * P, (k + 1) * P)
        dg = np.zeros((4, P, P), dtype=BF16)
        dv = dvals[sl]
        for pos in range(4):
            np.fill_diagonal(dg[pos], dv[:, pos].astype(BF16))
        per_core.append({
            "x_bf": np.ascontiguousarray(dev_bf[sl]),
            "x_f8": np.ascontiguousarray(dev_f8[sl]),
            "idn": np.eye(P, dtype=BF16),
            "dgs": np.ascontiguousarray(
                dg.transpose(1, 0, 2).reshape(P, 4 * P)),
        })
    return (per_core, slot_bucket, fill, folds, sum_x, sel_sum)


_NC_CACHE = None
LAST_RESULTS = None


def kernel(inputs: np.ndarray, targets: np.ndarray) -> np.ndarray:
    global _NC_CACHE, LAST_RESULTS
    x = np.ascontiguousarray(np.asarray(inputs, dtype=np.float32))
    t = np.ascontiguousarray(np.asarray(targets).astype(np.int64))
    assert x.shape == (B, C), x.shape
    assert t.shape == (B,), t.shape

    (per_core, slot_bucket, fill, folds, sum_x, sel_sum) = _prep_inputs(x, t)

    if _NC_CACHE is None:
        _NC_CACHE = build_nc()
    nc = _NC_CACHE

    trace = bool(os.environ.get("BASS_TRACE"))
    if trace:
        _ensure_axon_ntff_hook()
    res = run_bass_kernel_spmd(nc, per_core, list(range(NCORES)), trace=trace)
    LAST_RESULTS = res

    # ---- host fold (f64) ----
    offs = np.concatenate([[0], np.cumsum(W_LIST)]).astype(int)
    lse_total = 0.0
    pen_total = 0.0
    for k, r in enumerate(res.results):
        sl = slice(k * P, (k + 1) * P)
        lse_total += float(np.asarray(r["lse_acc"], np.float64).sum())
        fl = folds[sl]                          # [P, 3]
        ta = np.asarray(r["ttr_acc"], np.float64)
        ua = np.asarray(r["u_acc"], np.float64)
        ti = 0
        ui = 0
        for n in range(TILES):
            if PEN_FLAGS[n] == "ttr":
                pen_total += float((fl * ta[:, 3 * ti:3 * ti + 3]).sum())
                ti += 1
            else:
                pen_total += float(ua[:, ui].sum())
                ui += 1

    # ---- pad-row corrections ----
    # pad count per (slot, tile): overlap of [fill_s, RPP) with tile range
    lo = np.maximum(offs[:-1][None, :], fill[:, None])       # [S, T]
    np_st = np.maximum(0, offs[1:][None, :] - lo)            # pads per slot/tile
    # device constants for a zero row, per tile flavor
    for n in range(TILES):
        pads_b = np.zeros(C)
        for b in range(C):
            pads_b[b] = np_st[slot_bucket == b, n].sum()
        if EXP_FLAGS[n] == "dve":
            v = float(_schr_np(np.float32(0.0)))
        else:
            v = 1.0
        S_pad = 5.0 * v
        lnS_bf = float(np.float32(np.log(S_pad)).astype(BF16))
        rb_pad = float(_schr_np(np.float32(-lnS_bf)))
        lse_total -= pads_b.sum() * np.log(S_pad)
        pen_total -= float((pads_b * T_ROWSUM).sum()) * v * rb_pad

    ce_sum = lse_total - SMOOTH_ALL * sum_x - SMOOTH_OFF * sel_sum
    loss = (ce_sum + TPEN * pen_total) / B
    return np.float32(loss)


def _ensure_axon_ntff_hook():
    """Provide antenv.axon_hooks if the image lacks it (profiling only)."""
    import importlib
    try:
        importlib.import_module("antenv.axon_hooks")
        return
    except ImportError:
        pass
    import types
    mod = types.ModuleType("antenv.axon_hooks")
    mod._hook = None

    def set_axon_ntff_profile_hook(h):
        mod._hook = h

    def get_axon_ntff_profile_hook():
        if mod._hook is None:
            try:
                from trn_agent_boot.trn_boot import _ntff_profile_via_ctypes
                mod._hook = _ntff_profile_via_ctypes("/opt/axon/libaxon_pjrt.so")
            except Exception:
                mod._hook = None
        return mod._hook

    mod.set_axon_ntff_profile_hook = set_axon_ntff_profile_hook
    mod.get_axon_ntff_profile_hook = get_axon_ntff_profile_hook
    sys.modules["antenv.axon_hooks"] = mod
    try:
        import antenv
        antenv.axon_hooks = mod
    except ImportError:
        pass


# revision 9
# speedup vs baseline: 1.1309x; 1.1309x over previous
# Trainium2 Bass kernel for BloomStageLoss:
#   loss = mean(label-smoothing CE) + 0.1 * mean(transition penalty)
# over inputs [B, 5] f32, targets [B] int.  B = 4194304, 8 NeuronCores.
#
# Host-side stable sort of rows by target class with bucket-pure
# (core, partition) slots, plus a per-slot class-position permutation
# that puts the diagonal class (T[b,b]=0) last so the penalty pass can
# skip it entirely.
#
# Device math per row i (bucket b, softmax P = e/S):
#   lse_i  = ln S_i,  S_i = sum_c e_ic
#   pen_i  = sum_c T[b,c] P_ic   (diagonal class contributes 0)
# Device pipeline per tile (pos-blocked layout [P, 5*wn], all bf16 in):
#   exp:  DVE 1-instr Schraudolph (bf16 -> int16 RNE, bits are bf16 of
#         e^x) or ACT exp for one tile (ACT has slack)
#   S:    5 accumulating identity matmuls -> PSUM f32
#   Ln:   ACT psS -> lnS bf16 + accum lse
#   rb:   DVE Schraudolph exp(-lnS) ~ 1/S  (1 instr)
#   pen:  either U-matmuls (4 per chunk, diag(T) stationary; U.rb via
#         custom TTR from PSUM) or 3 skip-diag TTRs on e directly.
# A dummy-matmul warmup spin flips the PE HAM clock gate to 2.4 GHz
# before the first real matmul.  Host folds: exact f64 linear terms,
# per-(partition,pos) T-fold of the TTR accums, analytic pad-row
# corrections.

import os
import sys

sys.path.insert(0, "/opt/trn_rl_repo")

import numpy as np
import ml_dtypes
from contextlib import ExitStack

import concourse.bass as bass
import concourse.bacc as bacc
import concourse.tile as tile
from concourse import mybir
from concourse.bass_utils import run_bass_kernel_spmd
from concourse.dve_ops import TENSOR_TENSOR_REDUCE as TTR_OP

NCORES = 8
C = 5
P = 128
B = 4194304
RPP = 4160                      # rows per partition (slot size)
NSLOTS = NCORES * P             # 1024
CAP = NSLOTS * RPP              # 4259840
W_LIST = [256, 512, 1024, 1024, 512, 416, 256, 160]
assert sum(W_LIST) == RPP
TILES = len(W_LIST)
# per-tile engine flags (tunable):
EXP_FLAGS = ["dve", "dve", "act", "dve", "dve", "dve", "dve", "dve"]
PEN_FLAGS = ["ttr", "umm", "umm", "umm", "ttr", "umm", "ttr", "umm"]
N_WARMUP_MM = 16                # dummy matmuls to flip the PE HAM gate
# which hardware DGE ring (trigger engine) each x-tile DMA uses
DMA_ENG = ["sync"] * 8
SMOOTH_OFF = 0.875              # 1 - SMOOTHING - SMOOTHING/(C-1)
SMOOTH_ALL = 0.025              # SMOOTHING/(C-1)
TPEN = 0.1

_PHI = np.array([0.0, 0.5, 1.0, 2.0, 2.0], dtype=np.float64)
T_MAT = _PHI[np.abs(np.arange(C)[:, None] - np.arange(C)[None, :])]
T_ROWSUM = T_MAT.sum(axis=1)    # [5.5, 4, 3, 4, 5.5]

# class-position permutation per bucket: positions 0,1 singles, 2-3 the
# equal-T pair, 4 the diagonal class (skipped by the pen pass).
PERM = {
    0: [1, 2, 3, 4, 0],
    1: [3, 4, 0, 2, 1],
    2: [1, 3, 0, 4, 2],
    3: [0, 1, 2, 4, 3],
    4: [2, 3, 0, 1, 4],
}
PERM_ARR = np.array([PERM[b] for b in range(C)], dtype=np.int64)   # [5,5]
# fold coefficient per (bucket, pos): T[b, PERM[b][pos]]
FOLD_ARR = np.array(
    [[T_MAT[b, PERM[b][pos]] for pos in range(C)] for b in range(C)]
)
for b in range(C):
    assert FOLD_ARR[b, 2] == FOLD_ARR[b, 3], (b, FOLD_ARR[b])
    assert FOLD_ARR[b, 4] == 0.0

BF16 = ml_dtypes.bfloat16
FP8 = ml_dtypes.float8_e4m3fn

LOG2E = 1.4426950408889634
SCHR_A = 128.0 * LOG2E               # 184.665
SCHR_SHIFT = 7.25                    # mean-centers the sawtooth for N(0,1)
SCHR_B = 16256.0 - SCHR_SHIFT


def _schr_np(x):
    """Host replica of the device Schraudolph exp (bf16-bit space)."""
    i = np.rint(np.asarray(x, np.float32) * SCHR_A + SCHR_B).astype(np.int16)
    return i.view(BF16).astype(np.float64)


_TABLES_PATCHED = False


def _pin_act_tables():
    """Keep Exp/Ln only in their shared set so one ACT table load serves both."""
    global _TABLES_PATCHED
    if _TABLES_PATCHED:
        return
    import concourse.bacc as bacc_mod
    AF = mybir.ActivationFunctionType
    orig = bacc_mod.get_activation_tables

    def patched(arch):
        t = {k: set(v) for k, v in orig(arch).items()}
        both = [k for k, v in t.items() if AF.Exp in v and AF.Ln in v]
        if both:
            keep = both[0]
            for k, v in t.items():
                if k != keep:
                    v.discard(AF.Exp)
                    v.discard(AF.Ln)
        return t

    bacc_mod.get_activation_tables = patched
    _TABLES_PATCHED = True


N_TTR = sum(1 for f in PEN_FLAGS if f == "ttr")
N_UMM = TILES - N_TTR


def build_nc(ncores=NCORES):
    """Build + compile the single-core program (SPMD across ncores)."""
    _pin_act_tables()
    f32 = mybir.dt.float32
    bf16 = mybir.dt.bfloat16
    i16 = mybir.dt.int16
    fp8 = mybir.dt.float8e4
    AF = mybir.ActivationFunctionType
    ALU = mybir.AluOpType

    nc = bacc.Bacc("TRN2", target_bir_lowering=False, debug=False,
                   num_devices=ncores)
    wbf = 5 * sum(w for w, f in zip(W_LIST, EXP_FLAGS) if f == "dve")
    wf8 = 5 * sum(w for w, f in zip(W_LIST, EXP_FLAGS) if f == "act")
    xbf_d = nc.dram_tensor("x_bf", [P, max(wbf, 1)], bf16,
                           kind="ExternalInput").ap()
    xf8_d = nc.dram_tensor("x_f8", [P, max(wf8, 1)], fp8,
                           kind="ExternalInput").ap()
    idn_d = nc.dram_tensor("idn", [P, P], bf16, kind="ExternalInput").ap()
    dgs_d = nc.dram_tensor("dgs", [P, 4 * P], bf16, kind="ExternalInput").ap()
    lse_d = nc.dram_tensor("lse_acc", [P, TILES], f32,
                           kind="ExternalOutput").ap()
    ttr_d = nc.dram_tensor("ttr_acc", [P, max(3 * N_TTR, 1)], f32,
                           kind="ExternalOutput").ap()
    u_d = nc.dram_tensor("u_acc", [P, max(N_UMM, 1)], f32,
                         kind="ExternalOutput").ap()

    with tile.TileContext(nc) as tc, ExitStack() as ctx:
        xpool = ctx.enter_context(tc.tile_pool(name="xp", bufs=1))
        epool = ctx.enter_context(tc.tile_pool(name="ep", bufs=1))
        lpool = ctx.enter_context(tc.tile_pool(name="lp", bufs=1))
        rpool = ctx.enter_context(tc.tile_pool(name="rp", bufs=1))
        spool = ctx.enter_context(tc.tile_pool(name="sp", bufs=1))
        cpool = ctx.enter_context(tc.tile_pool(name="cp", bufs=1))
        apool = ctx.enter_context(tc.tile_pool(name="ap", bufs=1))
        psS_pool = ctx.enter_context(tc.tile_pool(name="psS", bufs=2,
                                                  space="PSUM"))
        psU_pool = ctx.enter_context(tc.tile_pool(name="psU", bufs=2,
                                                  space="PSUM"))

        lse_acc = apool.tile([P, TILES], f32)
        ttr_acc = apool.tile([P, max(3 * N_TTR, 1)], f32)
        u_acc = apool.tile([P, max(N_UMM, 1)], f32)

        bf_off = [0]
        f8_off = [0]
        for w, f in zip(W_LIST, EXP_FLAGS):
            bf_off.append(bf_off[-1] + (5 * w if f == "dve" else 0))
            f8_off.append(f8_off[-1] + (5 * w if f == "act" else 0))

        xts = [None] * TILES
        ets = [None] * TILES
        psSs = [None] * TILES
        psUs = [None] * TILES
        rbs = [None] * TILES
        ttr_idx = 0
        umm_idx = 0

        def dma_tile(n):
            wn = W_LIST[n]
            eng = nc.sync if DMA_ENG[n] == "sync" else nc.scalar
            if EXP_FLAGS[n] == "dve":
                xt = xpool.tile([P, 5 * wn], bf16, tag=f"xb{n}", name="xt")
                eng.dma_start(xt[:], xbf_d[:, bf_off[n]:bf_off[n + 1]])
            else:
                xt = xpool.tile([P, 5 * wn], fp8, tag=f"x8{n}", name="xt")
                eng.dma_start(xt[:], xf8_d[:, f8_off[n]:f8_off[n + 1]])
            xts[n] = xt

        # constants + ALL x tiles triggered up front on two HWDGE rings;
        # per-tile dedicated buffers mean no consumer pacing.
        ident = cpool.tile([P, P], bf16)
        nc.sync.dma_start(ident[:], idn_d)
        for n in range(3):
            dma_tile(n)
        dgs = cpool.tile([P, 4 * P], bf16)
        nc.sync.dma_start(dgs[:], dgs_d)
        for n in range(3, TILES):
            dma_tile(n)

        # PE HAM warmup: dummy matmuls on ident into psS(0)'s region (the
        # real S-matmuls reset it with start=True afterwards).
        psS0 = psS_pool.tile([P, W_LIST[0]], f32, tag="psS")
        psSs[0] = psS0
        with tc.high_priority(offset=300):
            for k in range(N_WARMUP_MM):
                nc.tensor.matmul(psS0[:, 0:P], ident[:], ident[:],
                                 start=(k == 0), stop=(k == N_WARMUP_MM - 1),
                                 skip_group_check=True)

        def exp_tile(n):
            """exp of tile n (engine per EXP_FLAGS)."""
            wn = W_LIST[n]
            xt = xts[n]
            if EXP_FLAGS[n] == "dve":
                et_i = epool.tile([P, 5 * wn], i16, tag=f"ei{n}")
                with tc.high_priority(offset=100):
                    nc.vector.tensor_scalar(et_i[:], xt[:], SCHR_A, SCHR_B,
                                            ALU.mult, ALU.add)
                ets[n] = et_i[:].bitcast(bf16)
            else:
                et_t = epool.tile([P, 5 * wn], bf16, tag=f"eb{n}")
                x3 = xt[:].rearrange("p (c w) -> p c w", c=C)
                e3 = et_t[:].rearrange("p (c w) -> p c w", c=C)
                for j0 in range(0, wn, 512):
                    j1 = min(j0 + 512, wn)
                    nc.scalar.activation(e3[:, :, j0:j1], x3[:, :, j0:j1],
                                         AF.Exp)
                ets[n] = et_t[:]

        def smm_tile(n):
            """S-matmuls (+U-matmuls) of tile n."""
            wn = W_LIST[n]
            et = ets[n]
            if psSs[n] is None:
                psSs[n] = psS_pool.tile([P, wn], f32, tag="psS", name="psS")
            psS = psSs[n]
            for j0 in range(0, wn, 512):
                j1 = min(j0 + 512, wn)
                with tc.high_priority(offset=200):
                    for pos in range(C):
                        nc.tensor.matmul(psS[:, j0:j1], ident[:],
                                         et[:, pos * wn + j0:pos * wn + j1],
                                         start=(pos == 0), stop=(pos == C - 1))
            if PEN_FLAGS[n] == "umm":
                psU = psU_pool.tile([P, wn], f32, tag="psU")
                psUs[n] = psU
                for j0 in range(0, wn, 512):
                    j1 = min(j0 + 512, wn)
                    for pos in range(4):
                        nc.tensor.matmul(psU[:, j0:j1],
                                         dgs[:, pos * P:(pos + 1) * P],
                                         et[:, pos * wn + j0:pos * wn + j1],
                                         start=(pos == 0), stop=(pos == 3))

        def front(n):
            exp_tile(n)
            smm_tile(n)

        def mid(n):
            """Ln + rexp of tile n."""
            wn = W_LIST[n]
            lnS = lpool.tile([P, wn], bf16, tag=f"ln{n}")
            nc.scalar.activation(lnS[:], psSs[n][:], AF.Ln,
                                 accum_out=lse_acc[:, n:n + 1])
            rb_i = rpool.tile([P, wn], i16, tag=f"rb{n}")
            with tc.high_priority(offset=100):
                nc.vector.tensor_scalar(rb_i[:], lnS[:], -SCHR_A, SCHR_B,
                                        ALU.mult, ALU.add)
            rbs[n] = rb_i[:].bitcast(bf16)

        def tail(n):
            """pen accumulation of tile n."""
            nonlocal ttr_idx, umm_idx
            wn = W_LIST[n]
            et = ets[n]
            rb = rbs[n]
            if PEN_FLAGS[n] == "umm":
                scr = spool.tile([P, wn], bf16, tag="scU")
                nc.vector._custom_dve(
                    TTR_OP, out=scr[:], in0=psUs[n][:], in1=rb,
                    s0=0.0, s1=1.0,
                    accum_out=u_acc[:, umm_idx:umm_idx + 1])
                umm_idx += 1
            else:
                k = 3 * ttr_idx
                scr = spool.tile([P, 2 * wn], bf16, tag="scT")
                for pos in range(2):
                    nc.vector._custom_dve(
                        TTR_OP, out=scr[:, pos * wn:(pos + 1) * wn],
                        in0=et[:, pos * wn:(pos + 1) * wn], in1=rb,
                        s0=0.0, s1=1.0,
                        accum_out=ttr_acc[:, k + pos:k + pos + 1])
                # pair: positions 2-3 share the fold coefficient
                e3 = et[:, 2 * wn:4 * wn].rearrange("p (c w) -> p c w", c=2)
                r3 = rb.unsqueeze(1).broadcast_to([P, 2, wn])
                s3 = scr[:].rearrange("p (c w) -> p c w", c=2)
                nc.vector._custom_dve(
                    TTR_OP, out=s3, in0=e3, in1=r3,
                    s0=0.0, s1=1.0,
                    accum_out=ttr_acc[:, k + 2:k + 3])
                ttr_idx += 1

        # software pipeline (DMAs already queued)
        front(0)
        mid(0)
        front(1)
        tail(0)
        mid(1)
        front(2)
        tail(1)
        mid(2)
        front(3)
        tail(2)
        mid(3)
        front(4)
        tail(3)
        mid(4)
        front(5)
        tail(4)
        mid(5)
        front(6)
        tail(5)
        mid(6)
        front(7)
        tail(6)
        mid(7)
        tail(7)

        nc.sync.dma_start(lse_d, lse_acc[:])
        nc.sync.dma_start(ttr_d, ttr_acc[:])
        nc.sync.dma_start(u_d, u_acc[:])

    nc.compile()
    return nc


def _prep_inputs(x: np.ndarray, t: np.ndarray):
    """Sort rows by target, pad buckets to slot (RPP) multiples, apply
    per-slot class-position permutation, lay out pos-blocked per tile."""
    counts = np.bincount(t, minlength=C).astype(np.int64)
    order = np.argsort(t, kind="stable")
    xs = x[order]                               # [B, 5] f32, bucket-contiguous

    # exact host-side sums (f64)
    sum_x = float(x.sum(dtype=np.float64))
    sel_sum = 0.0
    cstart = np.concatenate([[0], np.cumsum(counts)])
    for b in range(C):
        sel_sum += float(xs[cstart[b]:cstart[b + 1], b].sum(dtype=np.float64))

    slots_b = np.ceil(counts / RPP).astype(np.int64)
    assert slots_b.sum() <= NSLOTS, (counts, slots_b)
    slot_start = np.concatenate([[0], np.cumsum(slots_b)])
    # slot -> bucket map; trailing unused slots assigned to bucket C-1
    slot_bucket = np.full(NSLOTS, C - 1, dtype=np.int64)
    for b in range(C):
        slot_bucket[slot_start[b]:slot_start[b + 1]] = b

    # fill count per slot (rows of real data in that slot)
    fill = np.zeros(NSLOTS, dtype=np.int64)
    for b in range(C):
        cnt = counts[b]
        for s in range(slot_start[b], slot_start[b + 1]):
            fill[s] = min(RPP, cnt)
            cnt -= fill[s]

    # padded array [CAP, 5], zero rows as pad
    xpad = np.zeros((CAP, C), dtype=np.float32)
    for b in range(C):
        dst0 = slot_start[b] * RPP
        xpad[dst0:dst0 + counts[b]] = xs[cstart[b]:cstart[b + 1]]

    # per-slot class permutation -> position-blocked
    x3 = xpad.reshape(NSLOTS, RPP, C)
    perm_idx = PERM_ARR[slot_bucket]            # [NSLOTS, 5]
    x3p = np.take_along_axis(x3, perm_idx[:, None, :], axis=2)

    # device layout per tile: [NSLOTS, 5*wn], pos-major
    offs = np.concatenate([[0], np.cumsum(W_LIST)]).astype(int)
    bf_parts, f8_parts = [], []
    for n, wn in enumerate(W_LIST):
        blk = x3p[:, offs[n]:offs[n + 1], :].transpose(0, 2, 1)  # [S, 5, wn]
        blk = blk.reshape(NSLOTS, C * wn)
        if EXP_FLAGS[n] == "dve":
            bf_parts.append(blk.astype(BF16))
        else:
            f8_parts.append(np.clip(blk, -15.0, 15.0).astype(FP8))
    dev_bf = (np.ascontiguousarray(np.concatenate(bf_parts, axis=1))
              if bf_parts else np.zeros((NSLOTS, 1), dtype=BF16))
    dev_f8 = (np.ascontiguousarray(np.concatenate(f8_parts, axis=1))
              if f8_parts else np.zeros((NSLOTS, 1), dtype=FP8))

    # per-slot diag values for U-matmuls (positions 0..3)
    dvals = FOLD_ARR[slot_bucket][:, :4]        # [NSLOTS, 4]
    # fold coefficients for ttr tiles: [NSLOTS, 3] (single, single, pair)
    folds = FOLD_ARR[slot_bucket][:, [0, 1, 2]]

    per_core = []
    for k in range(NCORES):
        sl = slice(k * P, (k + 1) * P)
        dg = np.zeros((4, P, P), dtype=BF16)
        dv = dvals[sl]
        for pos in range(4):
            np.fill_diagonal(dg[pos], dv[:, pos].astype(BF16))
        per_core.append({
            "x_bf": np.ascontiguousarray(dev_bf[sl]),
            "x_f8": np.ascontiguousarray(dev_f8[sl]),
            "idn": np.eye(P, dtype=BF16),
            "dgs": np.ascontiguousarray(
                dg.transpose(1, 0, 2).reshape(P, 4 * P)),
        })
    return (per_core, slot_bucket, fill, folds, sum_x, sel_sum)


_NC_CACHE = None
LAST_RESULTS = None


def kernel(inputs: np.ndarray, targets: np.ndarray) -> np.ndarray:
    global _NC_CACHE, LAST_RESULTS
    x = np.ascontiguousarray(np.asarray(inputs, dtype=np.float32))
    t = np.ascontiguousarray(np.asarray(targets).astype(np.int64))
    assert x.shape == (B, C), x.shape
    assert t.shape == (B,), t.shape

    (per_core, slot_bucket, fill, folds, sum_x, sel_sum) = _prep_inputs(x, t)

    if _NC_CACHE is None:
        _NC_CACHE = build_nc()
    nc = _NC_CACHE

    trace = bool(os.environ.get("BASS_TRACE"))
    if trace:
        _ensure_axon_ntff_hook()
    res = run_bass_kernel_spmd(nc, per_core, list(range(NCORES)), trace=trace)
    LAST_RESULTS = res

    # ---- host fold (f64) ----
    offs = np.concatenate([[0], np.cumsum(W_LIST)]).astype(int)
    lse_total = 0.0
    pen_total = 0.0
    for k, r in enumerate(res.results):
        sl = slice(k * P, (k + 1) * P)
        lse_total += float(np.asarray(r["lse_acc"], np.float64).sum())
        fl = folds[sl]                          # [P, 3]
        ta = np.asarray(r["ttr_acc"], np.float64)
        ua = np.asarray(r["u_acc"], np.float64)
        ti = 0
        ui = 0
        for n in range(TILES):
            if PEN_FLAGS[n] == "ttr":
                pen_total += float((fl * ta[:, 3 * ti:3 * ti + 3]).sum())
                ti += 1
            else:
                pen_total += float(ua[:, ui].sum())
                ui += 1

    # ---- pad-row corrections ----
    # pad count per (slot, tile): overlap of [fill_s, RPP) with tile range
    lo = np.maximum(offs[:-1][None, :], fill[:, None])       # [S, T]
    np_st = np.maximum(0, offs[1:][None, :] - lo)            # pads per slot/tile
    # device constants for a zero row, per tile flavor
    for n in range(TILES):
        pads_b = np.zeros(C)
        for b in range(C):
            pads_b[b] = np_st[slot_bucket == b, n].sum()
        if EXP_FLAGS[n] == "dve":
            v = float(_schr_np(np.float32(0.0)))
        else:
            v = 1.0
        S_pad = 5.0 * v
        lnS_bf = float(np.float32(np.log(S_pad)).astype(BF16))
        rb_pad = float(_schr_np(np.float32(-lnS_bf)))
        lse_total -= pads_b.sum() * np.log(S_pad)
        pen_total -= float((pads_b * T_ROWSUM).sum()) * v * rb_pad

    ce_sum = lse_total - SMOOTH_ALL * sum_x - SMOOTH_OFF * sel_sum
    loss = (ce_sum + TPEN * pen_total) / B
    return np.float32(loss)


def _ensure_axon_ntff_hook():
    """Provide antenv.axon_hooks if the image lacks it (profiling only)."""
    import importlib
    try:
        importlib.import_module("antenv.axon_hooks")
        return
    except ImportError:
        pass
    import types
    mod = types.ModuleType("antenv.axon_hooks")
    mod._hook = None

    def set_axon_ntff_profile_hook(h):
        mod._hook = h

    def get_axon_ntff_profile_hook():
        if mod._hook is None:
            try:
                from trn_agent_boot.trn_boot import _ntff_profile_via_ctypes
                mod._hook = _ntff_profile_via_ctypes("/opt/axon/libaxon_pjrt.so")
            except Exception:
                mod._hook = None
        return mod._hook

    mod.set_axon_ntff_profile_hook = set_axon_ntff_profile_hook
    mod.get_axon_ntff_profile_hook = get_axon_ntff_profile_hook
    sys.modules["antenv.axon_hooks"] = mod
    try:
        import antenv
        antenv.axon_hooks = mod
    except ImportError:
        pass
